# revision 49
# baseline (speedup 1.0000x reference)
"""Causal self-attention layer (B=4, T=2048, C=1024, H=16) on 8 TRN2 NeuronCores.

Sharding: Megatron-style tensor parallel over heads — 2 heads per core.
Each core computes q/k/v projections for its 2 heads, causal flash-style
attention with ones-columns on V to accumulate softmax denominators, and a
partial output projection against its 128-row slice of W_proj. The host sums
the 8 partial projections and adds b_proj.

v2 over the 345us baseline:
- Score PAIRS: the two heads' score matmuls (K=64) are emitted back-to-back
  with disjoint row groups (h0 rows 0-63 via base_partition 0, h1 rows 64-127
  via base_partition 64) into the two banks of one [128,1024] PSUM tile. The
  PE runs them concurrently (tile_position auto-derives from base partitions),
  roughly halving score streaming time.
- One exp per pair: ACT reads [128, u0:1024] across both PSUM banks in a
  single ACTIVATE, halving the ~293ns per-instruction overhead. ACT does
  exp (and 1/3 of proj copies at qb boundaries) and nothing else: the q
  scale 1/sqrt(dh) is folded into W_q on the host, so qkv PSUM->SBUF copies
  are DVE tensor_scalar_add.
- Masks moved to GpSimd (SBUF-only tensor_mul) to unload DVE.
- HAM warmup: dummy matmuls at t=0 during the DMA prologue so the PE clock
  gate is at 8/8 when real matmuls start (baseline lost ~18us cold).
- Fine-grained fillers: qkv tiles / V transposes / prev-batch projection
  emit as generators pulled ~2 matmuls per attention pipeline step, keeping
  the PE dense while ACT paces the S->exp->PV chain.
- PSUM: spair 2x[128,1024] (4 banks) + py 2x[128,512] + filler 2x[128,512].
"""
import sys

sys.path.insert(0, "/opt/trn_rl_repo")

import numpy as np
import ml_dtypes

import concourse.bass as bass  # noqa: F401
from concourse import bacc
import concourse.mybir as mybir
import concourse.tile as tile
from concourse.bass_utils import run_bass_kernel_spmd
from concourse.masks import make_identity

B, T, C = 4, 2048, 1024
H, DH = 16, 64
N_CORES = 8
HPC = H // N_CORES          # heads per core = 2
DPC = HPC * DH              # head-dims per core = 128
NT = B * T                  # 8192 tokens
CH = C // 128               # 8 contraction chunks
QB = 512                    # q-block width (moving dim)
KT = 128                    # k-tile width (PE partition dim)
SCALE = 1.0 / 8.0           # 1/sqrt(DH) (folded into W_q on host)
TPB = T // QB               # qkv token tiles per batch = 4

F32 = mybir.dt.float32
BF16 = mybir.dt.bfloat16
AF = mybir.ActivationFunctionType
BF16_NP = ml_dtypes.bfloat16

_CACHED_NC = None
LAST_RESULT = None


def _build():
    nc = bacc.Bacc(None)

    xT = nc.dram_tensor("xT", [C, NT], BF16, kind="ExternalInput")
    # qkv weights pre-arranged on host to the SBUF layout [p, c, n]
    wq = nc.dram_tensor("wq", [128, CH, DPC], BF16, kind="ExternalInput")
    wk = nc.dram_tensor("wk", [128, CH, DPC], BF16, kind="ExternalInput")
    wv = nc.dram_tensor("wv", [128, CH, DPC], BF16, kind="ExternalInput")
    bqkv = nc.dram_tensor("bqkv", [DPC, 3], F32, kind="ExternalInput")
    wp = nc.dram_tensor("wp", [DPC, C], BF16, kind="ExternalInput")
    out = nc.dram_tensor("out", [NT, C], BF16, kind="ExternalOutput")

    with tile.TileContext(nc) as tc:
        with (
            tc.tile_pool(name="const", bufs=1) as const,
            tc.tile_pool(name="res", bufs=1) as res,
        ):
            # --- constants (built in f32, cast to bf16 once) ---
            ident = const.tile([128, 128], BF16, tag="ident")
            # sliding causal mask: wmask[k, u] = 1 iff k <= u - 512; a crossing
            # tile r multiplies by wmask[:, 512-128r : 1024-128r]
            wmask = const.tile([128, 1024], BF16, tag="wmask")
            ones_col = const.tile([128, 1], BF16, tag="ones_col")
            # lhsT for the K=1 outer-product that broadcasts a denominator
            # reciprocal row into a [64, QB] block of the normalizer
            ones_row = const.tile([1, 64], BF16, tag="ones_row")
            warm = const.tile([128, 640], BF16, tag="warm")
            nc.vector.memset(warm[:], 0.0)

            with tc.tile_pool(name="cstage", bufs=1) as cstage:
                ident_s = cstage.tile([128, 128], F32, tag="ident_s")
                make_identity(nc, ident_s[:])
                nc.vector.tensor_copy(ident[:], ident_s[:])

                wmask_s = cstage.tile([128, 1024], F32, tag="wmask_s")
                nc.gpsimd.memset(wmask_s[:], 0.0)
                nc.gpsimd.affine_select(
                    out=wmask_s[:],
                    in_=wmask_s[:],
                    compare_op=mybir.AluOpType.is_gt,
                    fill=1.0,
                    base=512,
                    # keep 0 where (512 + k - u) > 0, fill 1 where k <= u - 512
                    pattern=[[-1, 1024]],
                    channel_multiplier=1,
                )
                nc.vector.tensor_copy(wmask[:], wmask_s[:])

                ones_s = cstage.tile([128, 1], F32, tag="ones_s")
                nc.gpsimd.memset(ones_s[:], 1.0)
                nc.vector.tensor_copy(ones_col[:], ones_s[:])

                ones_rs = cstage.tile([1, 64], F32, tag="ones_rs")
                nc.gpsimd.memset(ones_rs[:], 1.0)
                nc.vector.tensor_copy(ones_row[:], ones_rs[:])

            bqkv_t = const.tile([DPC, 3], F32, tag="bqkv")
            bq_t, bk_t, bv_t = bqkv_t[:, 0:1], bqkv_t[:, 1:2], bqkv_t[:, 2:3]

            # weights -> SBUF directly in bf16 (cast on host)
            wq_r = const.tile([128, CH, DPC], BF16, tag="wq_r")
            wk_r = const.tile([128, CH, DPC], BF16, tag="wk_r")
            wv_r = const.tile([128, CH, DPC], BF16, tag="wv_r")
            wp_r = const.tile([DPC, C], BF16, tag="wp_r")

            # --- residents ---
            qT = res.tile([DPC, NT], BF16, tag="qT")
            kT = res.tile([DPC, NT], BF16, tag="kT")
            vT = res.tile([DPC, NT], BF16, tag="vT")
            yT = res.tile([DPC, NT], BF16, tag="yT")

            xT_re = xT.rearrange("(c p) t -> p c t", p=128)
            n_ktiles = T // KT  # 16

            with (
                tc.tile_pool(name="xpool", bufs=3) as xpool,
                tc.tile_pool(name="vpool", bufs=34) as vpool,
                tc.tile_pool(name="epool", bufs=4) as epool,
                tc.tile_pool(name="dpool", bufs=2) as dpool,
                tc.tile_pool(name="opool", bufs=8) as opool,
                tc.tile_pool(name="sp_psum", bufs=2, space="PSUM") as sp_psum,
                tc.tile_pool(name="py_psum", bufs=1, space="PSUM") as py_psum,
                tc.tile_pool(name="f_psum", bufs=2, space="PSUM") as f_psum,
            ):
                # ---------- HAM warmup: dummy matmuls during DMA wait ----------
                for i in range(13):
                    pw = f_psum.tile([128, QB], F32, tag="f", name=f"pw{i}")
                    nc.tensor.matmul(
                        pw[:], warm[:, :128], warm[:, 128:640],
                        start=True, stop=True,
                    )

                xs_tiles = {}

                def dma_x(tt, nsplit=2):
                    if tt >= NT // QB or tt in xs_tiles:
                        return
                    xs = xpool.tile([128, CH, QB], BF16, tag="xs", name=f"xs{tt}")
                    step = CH // nsplit
                    for c0 in range(0, CH, step):
                        nc.sync.dma_start(
                            xs[:, c0 : c0 + step, :],
                            xT_re[:, c0 : c0 + step, tt * QB : (tt + 1) * QB],
                        )
                    xs_tiles[tt] = xs

                def qkv_tile_gen(tt):
                    """Project one 512-token tile into qT/kT/vT; yields every
                    ~2 matmuls so it can be woven as filler."""
                    dma_x(tt + 1)
                    xs = xs_tiles.pop(tt)
                    ts_ = slice(tt * QB, (tt + 1) * QB)
                    outs = ((qT, bq_t, "q"), (kT, bk_t, "k"), (vT, bv_t, "v"))
                    for w_r, (dst, b_t, nm) in zip((wq_r, wk_r, wv_r), outs):
                        ps = f_psum.tile([128, QB], F32, tag="f", name=f"ps{nm}{tt}")
                        for c in range(CH):
                            nc.tensor.matmul(
                                ps[:], w_r[:, c, :], xs[:, c, :],
                                start=(c == 0), stop=(c == CH - 1),
                            )
                            if c % 2 == 1:
                                yield
                        nc.vector.tensor_scalar_add(dst[:, ts_], ps[:], b_t[:])

                # per-batch state
                vts_all = {}   # b -> list of 16 [128, 130] tiles
                den_all = {}   # (b, qb) -> SBUF den strip [1, 2*QB]

                def vts_block(b, quarter):
                    """Transpose 4 V token-tiles (both heads at once). Emitted
                    in small blocks at qb boundaries — transpose-mode doesn't
                    count as PE-busy for the HAM, so long transpose runs make
                    the clock gate re-throttle.

                    v tile layout [128 tok, 130]: cols 0-63 head0 dims, col 64
                    ones, cols 65-128 head1 dims, col 129 ones. Head hl's PV
                    lhsT is v[:, 65*hl : 65*hl+65] -> psum rows 0-63 = y,
                    row 64 = denominator.
                    """
                    cb = b * T
                    vts = vts_all.setdefault(b, [None] * n_ktiles)
                    for kt in range(quarter * 4, quarter * 4 + 4):
                        pt = f_psum.tile([128, QB], BF16, tag="f", name=f"pt{b}_{kt}")
                        nc.tensor.transpose(
                            pt[:, :128],
                            vT[:, cb + kt * KT : cb + (kt + 1) * KT],
                            ident[:],
                        )
                        v = vpool.tile([128, 130], BF16, tag="v", name=f"v{b}_{kt}")
                        nc.vector.tensor_copy(v[:, 0:64], pt[:, 0:64])
                        nc.vector.tensor_copy(v[:, 65:129], pt[:, 64:128])
                        nc.vector.tensor_copy(v[:, 64:65], ones_col[:])
                        nc.vector.tensor_copy(v[:, 129:130], ones_col[:])
                        vts[kt] = v

                # ---------- filler machinery ----------
                fill_q = []   # deque of active generators

                def pull(n):
                    """Emit ~n filler units (each ~2 matmuls)."""
                    for _ in range(n):
                        while fill_q:
                            try:
                                next(fill_q[0])
                                break
                            except StopIteration:
                                fill_q.pop(0)
                        else:
                            return

                def drain_fillers():
                    while fill_q:
                        try:
                            next(fill_q[0])
                        except StopIteration:
                            fill_q.pop(0)

                def sp_qb(b, qb, start_pull=3):
                    """Pipelined scores/exp/PV for one q-block, both heads.

                    PE order: S0,S1,[F,F,PV(j-2),S(j)]...,PV(n-2),PV(n-1)
                    ACT order: exp(0),exp(1),...  (one exp per head-pair)
                    """
                    cb = b * T
                    vts = vts_all[b]
                    qs = slice(cb + qb * QB, cb + (qb + 1) * QB)
                    nkt = (qb + 1) * (QB // KT)
                    # both heads' PV accumulators in ONE 2-bank tile so the
                    # denominator rows (row 64 of each bank) are contiguous
                    # for a single ACT copy
                    py = py_psum.tile([128, 2 * QB], F32, tag="py", name=f"py{b}_{qb}")
                    # diagonal tile r's first 128*r q-columns are fully masked:
                    # narrow S/exp/PV to [u0:]
                    u0s = {
                        kt: max(kt - qb * (QB // KT), 0) * KT for kt in range(nkt)
                    }
                    exs = {}

                    def s_pair(kt):
                        u0 = u0s[kt]
                        sp = sp_psum.tile([128, 2 * QB], F32, tag="sp", name=f"sp{kt}")
                        for hl in range(2):
                            rb = hl * DH
                            nc.tensor.matmul(
                                sp[:, hl * QB + u0 : (hl + 1) * QB],
                                kT[rb : rb + DH, cb + kt * KT : cb + (kt + 1) * KT],
                                qT[rb : rb + DH, cb + qb * QB + u0 : cb + (qb + 1) * QB],
                                start=True,
                                stop=True,
                            )
                        # one exp across both banks (the gap cols for diagonal
                        # tiles hold stale psum; those ex cols are never read)
                        ex = epool.tile([128, 2 * QB], BF16, tag="ex", name=f"ex{kt}")
                        nc.scalar.activation(ex[:, u0:], sp[:, u0:], AF.Exp)
                        r = kt - qb * (QB // KT)
                        if r >= 0:
                            # diagonal-crossing tile: zero out k > q. DVE —
                            # GpSimd is far too slow/laggy for this critical
                            # path (measured ~1.2us + 1.4us semaphore lag)
                            nc.vector.tensor_mul(
                                ex[:, u0:QB], ex[:, u0:QB],
                                wmask[:, 512 : 1024 - u0],
                            )
                            nc.vector.tensor_mul(
                                ex[:, QB + u0 :], ex[:, QB + u0 :],
                                wmask[:, 512 : 1024 - u0],
                            )
                        exs[kt] = ex

                    def pv_pair(kt):
                        u0 = u0s[kt]
                        ex = exs.pop(kt)
                        for hl in range(2):
                            nc.tensor.matmul(
                                py[: DH + 1, hl * QB + u0 : (hl + 1) * QB],
                                vts[kt][:, 65 * hl : 65 * hl + 65],
                                ex[:, hl * QB + u0 : (hl + 1) * QB],
                                start=(kt == 0),
                                stop=(kt == nkt - 1),
                            )

                    # pulls sit AFTER the S-pairs in the PE FIFO so filler
                    # matmuls cover the exp latency before the dependent PVs.
                    # Sized to exactly drain the batch's filler supply: a lean
                    # pull rate keeps fillers available through the whole
                    # batch (running dry late-batch drops PE duty and makes
                    # the HAM clock-gate re-throttle).
                    s_pair(0)
                    s_pair(1)
                    pull(start_pull)
                    for kt in range(2, nkt):
                        pv_pair(kt - 2)
                        s_pair(kt)
                        pull(1)
                    pv_pair(nkt - 2)
                    pull(1)
                    pv_pair(nkt - 1)

                    # den row to an SBUF strip right away (ACT + DVE halves in
                    # parallel, each within one PSUM bank; cross-partition
                    # writes only work to partition 0), then unnormalized y
                    den = dpool.tile([1, 2 * QB], F32, tag="den", name=f"den{b}_{qb}")
                    nc.scalar.copy(den[:, 0:QB], py[DH : DH + 1, 0:QB])
                    nc.vector.tensor_copy(den[:, QB : 2 * QB], py[DH : DH + 1, QB : 2 * QB])
                    nc.vector.tensor_copy(yT[0:DH, qs], py[:DH, 0:QB])
                    nc.vector.tensor_copy(yT[DH:DPC, qs], py[:DH, QB : 2 * QB])
                    den_all[(b, qb)] = den

                def norm_qb(b, qb, rec_on_act=False):
                    """Reciprocal + normalize for one q-block (both heads).

                    The denominator pair-row is read straight out of PSUM by
                    the DVE reciprocal (no DMA partition-scatter), then two
                    concurrent K=1 outer-product matmuls broadcast the
                    reciprocal rows into the [128, QB] per-dim normalizer.
                    """
                    den = den_all.pop((b, qb))
                    cb = b * T
                    qs = slice(cb + qb * QB, cb + (qb + 1) * QB)
                    recf = dpool.tile([1, 2 * QB], F32, tag="recf", name=f"recf{b}_{qb}")
                    nc.vector.reciprocal_approx_fast(recf[:], den[:])
                    rec = dpool.tile([1, 2 * QB], BF16, tag="rec", name=f"rec{b}_{qb}")
                    if rec_on_act:
                        nc.scalar.copy(rec[:], recf[:])
                    else:
                        nc.vector.tensor_copy(rec[:], recf[:])
                    pb = f_psum.tile([128, QB], F32, tag="f", name=f"pb{b}_{qb}")
                    for hl in range(2):
                        nc.tensor.matmul(
                            pb[hl * DH : (hl + 1) * DH, :],
                            ones_row[:],
                            rec[:, hl * QB : (hl + 1) * QB],
                            start=True, stop=True,
                            tile_position=(0, hl * DH),
                        )
                    nc.vector.tensor_mul(yT[:, qs], yT[:, qs], pb[:])

                def proj_gen(b, i, act_half=False):
                    """Output projection for 4 of the batch's 16 token tiles,
                    yielding per matmul. Copies alternate DVE/DVE/ACT; the
                    final (tail) quarter alternates DVE/ACT 1:1 since ACT has
                    no exps left there."""
                    cb = b * T
                    for tt in range(i * 4, i * 4 + 4):
                        trow = cb + tt * 128
                        for half in range(2):
                            pp = f_psum.tile([128, QB], F32, tag="f", name=f"pp{b}_{tt}_{half}")
                            nc.tensor.matmul(
                                pp[:],
                                yT[:, trow : trow + 128],
                                wp_r[:, half * QB : (half + 1) * QB],
                                start=True,
                                stop=True,
                            )
                            os_ = opool.tile([128, QB], BF16, tag="os", name=f"os{b}_{tt}_{half}")
                            n = tt * 2 + half
                            on_act = (n % 2 == 1) if act_half else (n % 3 == 2)
                            if on_act:
                                nc.scalar.copy(os_[:], pp[:])
                            else:
                                nc.vector.tensor_copy(os_[:], pp[:])
                            nc.sync.dma_start(
                                out[trow : trow + 128, half * QB : (half + 1) * QB],
                                os_[:],
                            )
                            yield

                # ================= emission schedule =================
                # DMA issue order: wq's first chunks + the first x tile gate
                # the first matmul, so they go first. HAM warmup matmuls
                # (already queued) cover the ~9us DMA-engine startup + the
                # transfer so real matmuls start at full clock.
                nc.sync.dma_start(wq_r[:, 0:2, :], wq[:, 0:2, :])
                dma_x(0, nsplit=4)
                nc.sync.dma_start(wq_r[:, 2:, :], wq[:, 2:, :])
                nc.sync.dma_start(wk_r[:], wk[:])
                nc.sync.dma_start(wv_r[:], wv[:])
                nc.sync.dma_start(bqkv_t[:], bqkv[:])
                nc.sync.dma_start(wp_r[:], wp[:])

                # batch 0 qkv + V transposes, dense (nothing to overlap yet);
                # transposes in 4-blocks after their source tiles land
                fill_q.append(qkv_tile_gen(0))
                fill_q.append(qkv_tile_gen(1))
                drain_fillers()
                vts_block(0, 0)
                vts_block(0, 1)
                fill_q.append(qkv_tile_gen(2))
                fill_q.append(qkv_tile_gen(3))
                drain_fillers()
                vts_block(0, 2)

                # proj(b, 3) generator handed to batch b+1's first boundary
                prev_last_proj = None
                for b in range(B):
                    last = b == B - 1
                    # in-step fillers: next batch's qkv tiles; for the last
                    # batch, the 2nd-to-last batch's quarters 0-2 (its own
                    # boundaries deliberately left them unconsumed)
                    if not last:
                        nb = (b + 1) * TPB
                        for tt in range(TPB):
                            fill_q.append(qkv_tile_gen(nb + tt))
                    else:
                        for i in range(3):
                            fill_q.append(proj_gen(b - 1, i))
                    # own quarters: batches 0/1 run them at their own qb
                    # boundaries right after the norm; batch 2 leaves them for
                    # batch 3's in-step fillers (b3 has no qkv work and is
                    # otherwise starved); batch 3 pushes each quarter onto the
                    # filler queue as its norm completes. own[3] always goes
                    # to the next batch's first boundary.
                    own = [
                        proj_gen(b, i, act_half=(last and i >= 2)) for i in range(4)
                    ]
                    own_at_bnd = b <= 1
                    # last batch: process the big q-block third so the final
                    # reciprocal chain (qb=2) hides under real matmuls
                    qb_order = [0, 1, 3, 2] if last else [0, 1, 2, 3]
                    normed = []

                    for j, qb in enumerate(qb_order):
                        sp_qb(b, qb, start_pull=2 if last else 3)
                        # qb boundary: ~8 projection matmuls (or filler pulls)
                        # cover the reciprocal latency before the pb matmul,
                        # plus 4 V transposes (HAM-invisible, spread thin)
                        g = None
                        if j == 0:
                            g = prev_last_proj
                        elif own_at_bnd:
                            g = own[normed[-1]]
                        if g is not None:
                            for _ in range(8):
                                next(g, None)
                        elif last:
                            pull(6 if j == 3 else 1)
                        else:
                            pull(2)
                        if j == 0:
                            vts_block(b, 3)
                            if b > 0:
                                vts_all.pop(b - 1, None)
                        elif not last:
                            vts_block(b + 1, j - 1)
                        norm_qb(b, qb, rec_on_act=last and j >= 2)
                        normed.append(qb)
                        if last:
                            fill_q.append(own[qb])
                    prev_last_proj = own[3]
                    drain_fillers()

    nc.compile()
    return nc


def _get_nc():
    global _CACHED_NC
    if _CACHED_NC is None:
        _CACHED_NC = _build()
    return _CACHED_NC


def kernel(x, W_qkv, b_qkv, W_proj, b_proj, _trace=False, _core_ids=None):
    global LAST_RESULT
    x = np.asarray(x, dtype=np.float32)
    W_qkv = np.asarray(W_qkv, dtype=np.float32)
    b_qkv = np.asarray(b_qkv, dtype=np.float32)
    W_proj = np.asarray(W_proj, dtype=np.float32)
    b_proj = np.asarray(b_proj, dtype=np.float32)

    xT = np.ascontiguousarray(x.reshape(NT, C).T).astype(BF16_NP)
    W_proj_b = W_proj.astype(BF16_NP)
    core_ids = list(range(N_CORES)) if _core_ids is None else _core_ids

    def w_pcn(col0, col1, scale=1.0):
        # [C, DPC] -> SBUF layout [p=128, c=CH, n=DPC], contiguous
        w = (W_qkv[:, col0:col1] * np.float32(scale)).astype(BF16_NP)
        w = w.reshape(CH, 128, DPC)
        return np.ascontiguousarray(w.transpose(1, 0, 2))

    in_maps = []
    for core in range(len(core_ids)):
        s = slice(core * DPC, (core + 1) * DPC)
        in_maps.append(
            {
                "xT": xT,
                # q scale folded into W_q (and its bias) on the host
                "wq": w_pcn(0 * C + core * DPC, 0 * C + (core + 1) * DPC, SCALE),
                "wk": w_pcn(1 * C + core * DPC, 1 * C + (core + 1) * DPC),
                "wv": w_pcn(2 * C + core * DPC, 2 * C + (core + 1) * DPC),
                "bqkv": np.ascontiguousarray(
                    np.stack(
                        [
                            b_qkv[0 * C + core * DPC : 0 * C + (core + 1) * DPC] * np.float32(SCALE),
                            b_qkv[1 * C + core * DPC : 1 * C + (core + 1) * DPC],
                            b_qkv[2 * C + core * DPC : 2 * C + (core + 1) * DPC],
                        ],
                        axis=1,
                    )
                ),
                "wp": np.ascontiguousarray(W_proj_b[s, :]),
            }
        )

    nc = _get_nc()
    res = run_bass_kernel_spmd(nc, in_maps, core_ids, trace=_trace)
    LAST_RESULT = res

    acc = np.zeros((NT, C), dtype=np.float64)
    for r in res.results:
        acc += r["out"].astype(np.float64)
    acc += b_proj.astype(np.float64)
    return acc.reshape(B, T, C).astype(np.float32)


# revision 50
# speedup vs baseline: 1.1917x; 1.1917x over previous
"""Causal self-attention layer (B=4, T=2048, C=1024, H=16) on 8 TRN2 NeuronCores.

Sharding: Megatron-style tensor parallel over heads — 2 heads per core.
Each core computes q/k/v projections for its 2 heads, causal flash-style
attention with ones-columns on V to accumulate softmax denominators, and a
partial output projection against its 128-row slice of W_proj. The host sums
the 8 partial projections and adds b_proj.

v2 over the 345us baseline (now ~338us):
- Score PAIRS: the two heads' score matmuls (K=64) are emitted back-to-back
  with disjoint row groups (h0 rows 0-63 via base_partition 0, h1 rows 64-127
  via base_partition 64) into the two banks of one [128,1024] PSUM tile. The
  PE runs them concurrently (verified Dstart 3-4ns in the trace), roughly
  halving score streaming time.
- One exp per pair: ACT reads [128, u0:1024] across both PSUM banks in a
  single ACTIVATE, halving the ~293ns per-instruction overhead. ACT does
  exp + den-row staging + 1/3 of proj copies and nothing else: the q scale
  1/sqrt(dh) is folded into W_q on the host, so qkv PSUM->SBUF copies are
  DVE tensor_scalar_add. Masks stay on DVE (GpSimd is ~1.2us/op + 1.4us
  semaphore lag — tried and reverted).
- No den DMA: the softmax denominator row (PSUM row 64 of each py bank) is
  staged to a [1, 2*QB] SBUF strip (ACT+DVE halves; engine cross-partition
  writes only work to partition 0, and recip cannot read PSUM or cross
  banks), recip'd there, and broadcast into the [128,QB] normalizer by two
  concurrent K=1 outer-product matmuls (ones_row lhsT, col positions 0/64).
- HAM management: 13 dummy matmuls at t=0 pre-warm the clock gate through
  the ~9us DMA-engine startup; V transposes (not HAM-visible) are spread in
  4-blocks across qb boundaries; filler pulls are sized so supply lasts the
  whole batch (PE-duty dips re-throttle the HAM in 3.4us quanta).
- Fine-grained fillers: next batch's qkv as generators pulled ~1 yield (2
  matmuls) per pipeline step; per-batch proj quarters at qb boundaries
  (8 matmuls covering each reciprocal chain); batch 2 leaves its quarters
  to feed the qkv-less last batch, which orders q-blocks [0,1,3,2] so the
  final chain hides under real work.
- PSUM: spair 2x[128,1024] (4 banks) + py 1x[128,1024] (2) + filler
  2x[128,512] (2).
The interleaving of batch-3 q-blocks into batch 2's stream was tried and
reverted: it intermittently corrupted outputs (unresolved race) and was no
faster. reciprocal_approx_fast misreads PSUM inputs (bank wrap) — always
stage to SBUF first.
"""
import sys

sys.path.insert(0, "/opt/trn_rl_repo")

import numpy as np
import ml_dtypes

import concourse.bass as bass  # noqa: F401
from concourse import bacc
import concourse.mybir as mybir
import concourse.tile as tile
from concourse.bass_utils import run_bass_kernel_spmd
from concourse.masks import make_identity

B, T, C = 4, 2048, 1024
H, DH = 16, 64
N_CORES = 8
HPC = H // N_CORES          # heads per core = 2
DPC = HPC * DH              # head-dims per core = 128
NT = B * T                  # 8192 tokens
CH = C // 128               # 8 contraction chunks
QB = 512                    # q-block width (moving dim)
KT = 128                    # k-tile width (PE partition dim)
SCALE = 1.0 / 8.0           # 1/sqrt(DH) (folded into W_q on host)
TPB = T // QB               # qkv token tiles per batch = 4

F32 = mybir.dt.float32
BF16 = mybir.dt.bfloat16
AF = mybir.ActivationFunctionType
BF16_NP = ml_dtypes.bfloat16

_CACHED_NC = None
LAST_RESULT = None


def _build():
    nc = bacc.Bacc(None)

    xT = nc.dram_tensor("xT", [C, NT], BF16, kind="ExternalInput")
    # qkv weights pre-arranged on host to the SBUF layout [p, c, n]
    wq = nc.dram_tensor("wq", [128, CH, DPC], BF16, kind="ExternalInput")
    wk = nc.dram_tensor("wk", [128, CH, DPC], BF16, kind="ExternalInput")
    wv = nc.dram_tensor("wv", [128, CH, DPC], BF16, kind="ExternalInput")
    bqkv = nc.dram_tensor("bqkv", [DPC, 3], F32, kind="ExternalInput")
    wp = nc.dram_tensor("wp", [DPC, C], BF16, kind="ExternalInput")
    out = nc.dram_tensor("out", [NT, C], BF16, kind="ExternalOutput")

    with tile.TileContext(nc) as tc:
        with (
            tc.tile_pool(name="const", bufs=1) as const,
            tc.tile_pool(name="res", bufs=1) as res,
        ):
            # --- constants (built in f32, cast to bf16 once) ---
            ident = const.tile([128, 128], BF16, tag="ident")
            # sliding causal mask: wmask[k, u] = 1 iff k <= u - 512; a crossing
            # tile r multiplies by wmask[:, 512-128r : 1024-128r]
            wmask = const.tile([128, 1024], BF16, tag="wmask")
            ones_col = const.tile([128, 1], BF16, tag="ones_col")
            # lhsT for the K=1 outer-product that broadcasts a denominator
            # reciprocal row into a [64, QB] block of the normalizer
            ones_row = const.tile([1, 64], BF16, tag="ones_row")
            warm = const.tile([128, 640], BF16, tag="warm")
            nc.vector.memset(warm[:], 0.0)

            with tc.tile_pool(name="cstage", bufs=1) as cstage:
                ident_s = cstage.tile([128, 128], F32, tag="ident_s")
                make_identity(nc, ident_s[:])
                nc.vector.tensor_copy(ident[:], ident_s[:])

                wmask_s = cstage.tile([128, 1024], F32, tag="wmask_s")
                nc.gpsimd.memset(wmask_s[:], 0.0)
                nc.gpsimd.affine_select(
                    out=wmask_s[:],
                    in_=wmask_s[:],
                    compare_op=mybir.AluOpType.is_gt,
                    fill=1.0,
                    base=512,
                    # keep 0 where (512 + k - u) > 0, fill 1 where k <= u - 512
                    pattern=[[-1, 1024]],
                    channel_multiplier=1,
                )
                nc.vector.tensor_copy(wmask[:], wmask_s[:])

                ones_s = cstage.tile([128, 1], F32, tag="ones_s")
                nc.gpsimd.memset(ones_s[:], 1.0)
                nc.vector.tensor_copy(ones_col[:], ones_s[:])

                ones_rs = cstage.tile([1, 64], F32, tag="ones_rs")
                nc.gpsimd.memset(ones_rs[:], 1.0)
                nc.vector.tensor_copy(ones_row[:], ones_rs[:])

            bqkv_t = const.tile([DPC, 3], F32, tag="bqkv")
            bq_t, bk_t, bv_t = bqkv_t[:, 0:1], bqkv_t[:, 1:2], bqkv_t[:, 2:3]

            # weights -> SBUF directly in bf16 (cast on host)
            wq_r = const.tile([128, CH, DPC], BF16, tag="wq_r")
            wk_r = const.tile([128, CH, DPC], BF16, tag="wk_r")
            wv_r = const.tile([128, CH, DPC], BF16, tag="wv_r")
            wp_r = const.tile([DPC, C], BF16, tag="wp_r")

            # --- residents ---
            qT = res.tile([DPC, NT], BF16, tag="qT")
            kT = res.tile([DPC, NT], BF16, tag="kT")
            vT = res.tile([DPC, NT], BF16, tag="vT")
            yT = res.tile([DPC, NT], BF16, tag="yT")

            xT_re = xT.rearrange("(c p) t -> p c t", p=128)
            n_ktiles = T // KT  # 16

            with (
                tc.tile_pool(name="xpool", bufs=3) as xpool,
                tc.tile_pool(name="vpool", bufs=34) as vpool,
                tc.tile_pool(name="epool", bufs=4) as epool,
                tc.tile_pool(name="dpool", bufs=2) as dpool,
                tc.tile_pool(name="opool", bufs=8) as opool,
                tc.tile_pool(name="sp_psum", bufs=2, space="PSUM") as sp_psum,
                tc.tile_pool(name="py_psum", bufs=1, space="PSUM") as py_psum,
                tc.tile_pool(name="f_psum", bufs=2, space="PSUM") as f_psum,
            ):
                # ---------- HAM warmup: dummy matmuls during DMA wait ----------
                for i in range(13):
                    pw = f_psum.tile([128, QB], F32, tag="f", name=f"pw{i}")
                    nc.tensor.matmul(
                        pw[:], warm[:, :128], warm[:, 128:640],
                        start=True, stop=True,
                    )

                xs_tiles = {}

                def dma_x(tt, nsplit=2):
                    if tt >= NT // QB or tt in xs_tiles:
                        return
                    xs = xpool.tile([128, CH, QB], BF16, tag="xs", name=f"xs{tt}")
                    step = CH // nsplit
                    for c0 in range(0, CH, step):
                        nc.sync.dma_start(
                            xs[:, c0 : c0 + step, :],
                            xT_re[:, c0 : c0 + step, tt * QB : (tt + 1) * QB],
                        )
                    xs_tiles[tt] = xs

                def qkv_tile_gen(tt):
                    """Project one 512-token tile into qT/kT/vT; yields every
                    ~2 matmuls so it can be woven as filler."""
                    dma_x(tt + 1)
                    xs = xs_tiles.pop(tt)
                    ts_ = slice(tt * QB, (tt + 1) * QB)
                    outs = ((qT, bq_t, "q"), (kT, bk_t, "k"), (vT, bv_t, "v"))
                    for w_r, (dst, b_t, nm) in zip((wq_r, wk_r, wv_r), outs):
                        ps = f_psum.tile([128, QB], F32, tag="f", name=f"ps{nm}{tt}")
                        for c in range(CH):
                            nc.tensor.matmul(
                                ps[:], w_r[:, c, :], xs[:, c, :],
                                start=(c == 0), stop=(c == CH - 1),
                            )
                            if c % 2 == 1:
                                yield
                        nc.vector.tensor_scalar_add(dst[:, ts_], ps[:], b_t[:])

                # per-batch state
                vts_all = {}   # b -> list of 16 [128, 130] tiles
                den_all = {}   # (b, qb) -> SBUF den strip [1, 2*QB]

                def vts_block(b, quarter):
                    """Transpose 4 V token-tiles (both heads at once). Emitted
                    in small blocks at qb boundaries — transpose-mode doesn't
                    count as PE-busy for the HAM, so long transpose runs make
                    the clock gate re-throttle.

                    v tile layout [128 tok, 130]: cols 0-63 head0 dims, col 64
                    ones, cols 65-128 head1 dims, col 129 ones. Head hl's PV
                    lhsT is v[:, 65*hl : 65*hl+65] -> psum rows 0-63 = y,
                    row 64 = denominator.
                    """
                    cb = b * T
                    vts = vts_all.setdefault(b, [None] * n_ktiles)
                    for kt in range(quarter * 4, quarter * 4 + 4):
                        pt = f_psum.tile([128, QB], BF16, tag="f", name=f"pt{b}_{kt}")
                        nc.tensor.transpose(
                            pt[:, :128],
                            vT[:, cb + kt * KT : cb + (kt + 1) * KT],
                            ident[:],
                        )
                        v = vpool.tile([128, 130], BF16, tag="v", name=f"v{b}_{kt}")
                        nc.vector.tensor_copy(v[:, 0:64], pt[:, 0:64])
                        nc.vector.tensor_copy(v[:, 65:129], pt[:, 64:128])
                        nc.vector.tensor_copy(v[:, 64:65], ones_col[:])
                        nc.vector.tensor_copy(v[:, 129:130], ones_col[:])
                        vts[kt] = v

                # ---------- filler machinery ----------
                fill_q = []   # deque of active generators

                def pull(n):
                    """Emit ~n filler units (each ~2 matmuls)."""
                    for _ in range(n):
                        while fill_q:
                            try:
                                next(fill_q[0])
                                break
                            except StopIteration:
                                fill_q.pop(0)
                        else:
                            return

                def drain_fillers():
                    while fill_q:
                        try:
                            next(fill_q[0])
                        except StopIteration:
                            fill_q.pop(0)

                def sp_qb(b, qb, start_pull=3):
                    """Pipelined scores/exp/PV for one q-block, both heads.

                    PE order: S0,S1,[F,F,PV(j-2),S(j)]...,PV(n-2),PV(n-1)
                    ACT order: exp(0),exp(1),...  (one exp per head-pair)
                    """
                    cb = b * T
                    vts = vts_all[b]
                    qs = slice(cb + qb * QB, cb + (qb + 1) * QB)
                    nkt = (qb + 1) * (QB // KT)
                    # both heads' PV accumulators in ONE 2-bank tile so the
                    # denominator rows (row 64 of each bank) are contiguous
                    # for a single ACT copy
                    py = py_psum.tile([128, 2 * QB], F32, tag="py", name=f"py{b}_{qb}")
                    # diagonal tile r's first 128*r q-columns are fully masked:
                    # narrow S/exp/PV to [u0:]
                    u0s = {
                        kt: max(kt - qb * (QB // KT), 0) * KT for kt in range(nkt)
                    }
                    exs = {}

                    def s_pair(kt):
                        u0 = u0s[kt]
                        sp = sp_psum.tile([128, 2 * QB], F32, tag="sp", name=f"sp{kt}")
                        for hl in range(2):
                            rb = hl * DH
                            nc.tensor.matmul(
                                sp[:, hl * QB + u0 : (hl + 1) * QB],
                                kT[rb : rb + DH, cb + kt * KT : cb + (kt + 1) * KT],
                                qT[rb : rb + DH, cb + qb * QB + u0 : cb + (qb + 1) * QB],
                                start=True,
                                stop=True,
                            )
                        # one exp across both banks (the gap cols for diagonal
                        # tiles hold stale psum; those ex cols are never read)
                        ex = epool.tile([128, 2 * QB], BF16, tag="ex", name=f"ex{kt}")
                        nc.scalar.activation(ex[:, u0:], sp[:, u0:], AF.Exp)
                        r = kt - qb * (QB // KT)
                        if r >= 0:
                            # diagonal-crossing tile: zero out k > q. DVE —
                            # GpSimd is far too slow/laggy for this critical
                            # path (measured ~1.2us + 1.4us semaphore lag)
                            nc.vector.tensor_mul(
                                ex[:, u0:QB], ex[:, u0:QB],
                                wmask[:, 512 : 1024 - u0],
                            )
                            nc.vector.tensor_mul(
                                ex[:, QB + u0 :], ex[:, QB + u0 :],
                                wmask[:, 512 : 1024 - u0],
                            )
                        exs[kt] = ex

                    def pv_pair(kt):
                        u0 = u0s[kt]
                        ex = exs.pop(kt)
                        for hl in range(2):
                            nc.tensor.matmul(
                                py[: DH + 1, hl * QB + u0 : (hl + 1) * QB],
                                vts[kt][:, 65 * hl : 65 * hl + 65],
                                ex[:, hl * QB + u0 : (hl + 1) * QB],
                                start=(kt == 0),
                                stop=(kt == nkt - 1),
                            )

                    # pulls sit AFTER the S-pairs in the PE FIFO so filler
                    # matmuls cover the exp latency before the dependent PVs.
                    # Sized to exactly drain the batch's filler supply: a lean
                    # pull rate keeps fillers available through the whole
                    # batch (running dry late-batch drops PE duty and makes
                    # the HAM clock-gate re-throttle).
                    s_pair(0)
                    s_pair(1)
                    pull(start_pull)
                    for kt in range(2, nkt):
                        pv_pair(kt - 2)
                        s_pair(kt)
                        pull(1)
                    pv_pair(nkt - 2)
                    pull(1)
                    pv_pair(nkt - 1)

                    # den row to an SBUF strip right away (ACT + DVE halves in
                    # parallel, each within one PSUM bank; cross-partition
                    # writes only work to partition 0), then unnormalized y
                    den = dpool.tile([1, 2 * QB], F32, tag="den", name=f"den{b}_{qb}")
                    nc.scalar.copy(den[:, 0:QB], py[DH : DH + 1, 0:QB])
                    nc.vector.tensor_copy(den[:, QB : 2 * QB], py[DH : DH + 1, QB : 2 * QB])
                    nc.vector.tensor_copy(yT[0:DH, qs], py[:DH, 0:QB])
                    nc.vector.tensor_copy(yT[DH:DPC, qs], py[:DH, QB : 2 * QB])
                    den_all[(b, qb)] = den

                def norm_qb(b, qb, rec_on_act=False):
                    """Reciprocal + normalize for one q-block (both heads).

                    The denominator pair-row is read straight out of PSUM by
                    the DVE reciprocal (no DMA partition-scatter), then two
                    concurrent K=1 outer-product matmuls broadcast the
                    reciprocal rows into the [128, QB] per-dim normalizer.
                    """
                    den = den_all.pop((b, qb))
                    cb = b * T
                    qs = slice(cb + qb * QB, cb + (qb + 1) * QB)
                    recf = dpool.tile([1, 2 * QB], F32, tag="recf", name=f"recf{b}_{qb}")
                    nc.vector.reciprocal_approx_fast(recf[:], den[:])
                    rec = dpool.tile([1, 2 * QB], BF16, tag="rec", name=f"rec{b}_{qb}")
                    if rec_on_act:
                        nc.scalar.copy(rec[:], recf[:])
                    else:
                        nc.vector.tensor_copy(rec[:], recf[:])
                    pb = f_psum.tile([128, QB], F32, tag="f", name=f"pb{b}_{qb}")
                    for hl in range(2):
                        nc.tensor.matmul(
                            pb[hl * DH : (hl + 1) * DH, :],
                            ones_row[:],
                            rec[:, hl * QB : (hl + 1) * QB],
                            start=True, stop=True,
                            tile_position=(0, hl * DH),
                        )
                    nc.vector.tensor_mul(yT[:, qs], yT[:, qs], pb[:])

                def proj_gen(b, i, act_half=False):
                    """Output projection for 4 of the batch's 16 token tiles,
                    yielding per matmul. Copies alternate DVE/DVE/ACT; the
                    final (tail) quarter alternates DVE/ACT 1:1 since ACT has
                    no exps left there."""
                    cb = b * T
                    for tt in range(i * 4, i * 4 + 4):
                        trow = cb + tt * 128
                        for half in range(2):
                            pp = f_psum.tile([128, QB], F32, tag="f", name=f"pp{b}_{tt}_{half}")
                            nc.tensor.matmul(
                                pp[:],
                                yT[:, trow : trow + 128],
                                wp_r[:, half * QB : (half + 1) * QB],
                                start=True,
                                stop=True,
                            )
                            os_ = opool.tile([128, QB], BF16, tag="os", name=f"os{b}_{tt}_{half}")
                            n = tt * 2 + half
                            on_act = (n % 2 == 1) if act_half else (n % 3 == 2)
                            if on_act:
                                nc.scalar.copy(os_[:], pp[:])
                            else:
                                nc.vector.tensor_copy(os_[:], pp[:])
                            nc.sync.dma_start(
                                out[trow : trow + 128, half * QB : (half + 1) * QB],
                                os_[:],
                            )
                            yield

                # ================= emission schedule =================
                # DMA issue order: wq's first chunks + the first x tile gate
                # the first matmul, so they go first. HAM warmup matmuls
                # (already queued) cover the ~9us DMA-engine startup + the
                # transfer so real matmuls start at full clock.
                nc.sync.dma_start(wq_r[:, 0:2, :], wq[:, 0:2, :])
                dma_x(0, nsplit=4)
                nc.sync.dma_start(wq_r[:, 2:, :], wq[:, 2:, :])
                nc.sync.dma_start(wk_r[:], wk[:])
                nc.sync.dma_start(wv_r[:], wv[:])
                nc.sync.dma_start(bqkv_t[:], bqkv[:])
                nc.sync.dma_start(wp_r[:], wp[:])

                # batch 0 qkv + V transposes, dense (nothing to overlap yet);
                # transposes in 4-blocks after their source tiles land
                fill_q.append(qkv_tile_gen(0))
                fill_q.append(qkv_tile_gen(1))
                drain_fillers()
                vts_block(0, 0)
                vts_block(0, 1)
                fill_q.append(qkv_tile_gen(2))
                fill_q.append(qkv_tile_gen(3))
                drain_fillers()
                vts_block(0, 2)

                # proj(b, 3) generator handed to batch b+1's first boundary
                prev_last_proj = None
                for b in range(B):
                    last = b == B - 1
                    # in-step fillers: next batch's qkv tiles; for the last
                    # batch, the 2nd-to-last batch's quarters 0-2 (its own
                    # boundaries deliberately left them unconsumed)
                    if not last:
                        nb = (b + 1) * TPB
                        for tt in range(TPB):
                            fill_q.append(qkv_tile_gen(nb + tt))
                    else:
                        for i in range(3):
                            fill_q.append(proj_gen(b - 1, i))
                    # own quarters: batches 0/1 run them at their own qb
                    # boundaries right after the norm; batch 2 leaves them for
                    # batch 3's in-step fillers (b3 has no qkv work and is
                    # otherwise starved); batch 3 pushes each quarter onto the
                    # filler queue as its norm completes. own[3] always goes
                    # to the next batch's first boundary.
                    own = [
                        proj_gen(b, i, act_half=(last and i >= 2)) for i in range(4)
                    ]
                    own_at_bnd = b <= 1
                    # last batch: process the big q-block third so the final
                    # reciprocal chain (qb=2) hides under real matmuls
                    qb_order = [0, 1, 3, 2] if last else [0, 1, 2, 3]
                    normed = []

                    for j, qb in enumerate(qb_order):
                        sp_qb(b, qb, start_pull=2 if last else 3)
                        # qb boundary: ~8 projection matmuls (or filler pulls)
                        # cover the reciprocal latency before the pb matmul,
                        # plus 4 V transposes (HAM-invisible, spread thin)
                        g = None
                        if j == 0:
                            g = prev_last_proj
                        elif own_at_bnd:
                            g = own[normed[-1]]
                        if g is not None:
                            for _ in range(8):
                                next(g, None)
                        elif last:
                            pull(6 if j == 3 else 1)
                        else:
                            pull(2)
                        if j == 0:
                            vts_block(b, 3)
                            if b > 0:
                                vts_all.pop(b - 1, None)
                        elif not last:
                            vts_block(b + 1, j - 1)
                        norm_qb(b, qb, rec_on_act=last and j >= 2)
                        normed.append(qb)
                        if last:
                            fill_q.append(own[qb])
                    prev_last_proj = own[3]
                    drain_fillers()

    nc.compile()
    return nc


def _get_nc():
    global _CACHED_NC
    if _CACHED_NC is None:
        _CACHED_NC = _build()
    return _CACHED_NC


def kernel(x, W_qkv, b_qkv, W_proj, b_proj, _trace=False, _core_ids=None):
    global LAST_RESULT
    x = np.asarray(x, dtype=np.float32)
    W_qkv = np.asarray(W_qkv, dtype=np.float32)
    b_qkv = np.asarray(b_qkv, dtype=np.float32)
    W_proj = np.asarray(W_proj, dtype=np.float32)
    b_proj = np.asarray(b_proj, dtype=np.float32)

    xT = np.ascontiguousarray(x.reshape(NT, C).T).astype(BF16_NP)
    W_proj_b = W_proj.astype(BF16_NP)
    core_ids = list(range(N_CORES)) if _core_ids is None else _core_ids

    def w_pcn(col0, col1, scale=1.0):
        # [C, DPC] -> SBUF layout [p=128, c=CH, n=DPC], contiguous
        w = (W_qkv[:, col0:col1] * np.float32(scale)).astype(BF16_NP)
        w = w.reshape(CH, 128, DPC)
        return np.ascontiguousarray(w.transpose(1, 0, 2))

    in_maps = []
    for core in range(len(core_ids)):
        s = slice(core * DPC, (core + 1) * DPC)
        in_maps.append(
            {
                "xT": xT,
                # q scale folded into W_q (and its bias) on the host
                "wq": w_pcn(0 * C + core * DPC, 0 * C + (core + 1) * DPC, SCALE),
                "wk": w_pcn(1 * C + core * DPC, 1 * C + (core + 1) * DPC),
                "wv": w_pcn(2 * C + core * DPC, 2 * C + (core + 1) * DPC),
                "bqkv": np.ascontiguousarray(
                    np.stack(
                        [
                            b_qkv[0 * C + core * DPC : 0 * C + (core + 1) * DPC] * np.float32(SCALE),
                            b_qkv[1 * C + core * DPC : 1 * C + (core + 1) * DPC],
                            b_qkv[2 * C + core * DPC : 2 * C + (core + 1) * DPC],
                        ],
                        axis=1,
                    )
                ),
                "wp": np.ascontiguousarray(W_proj_b[s, :]),
            }
        )

    nc = _get_nc()
    res = run_bass_kernel_spmd(nc, in_maps, core_ids, trace=_trace)
    LAST_RESULT = res

    acc = np.zeros((NT, C), dtype=np.float64)
    for r in res.results:
        acc += r["out"].astype(np.float64)
    acc += b_proj.astype(np.float64)
    return acc.reshape(B, T, C).astype(np.float32)
